# revision 9
# baseline (speedup 1.0000x reference)
"""Trainium2 Bass kernel for a 2-layer bidirectional projected LSTM encoder.

Problem: x (512, 128) int32 tokens -> embedding (30, 64) -> 2 layers of
bidirectional LSTM (hidden 128, proj 64) -> output (512, 128, 128) f32.

Strategy (per spec sharding hint): data-parallel over batch, 16 batch
elements per NeuronCore, weights replicated. Per core, the two directions
of a layer are fused into shared instructions (feature-major layout, 128
partitions = hidden unit, free dim = [gate|dir|batch]).

Key device-side structure per layer:
  - "gx" input contributions Wih@x + bias are computed by batched matmuls
    directly into per-timestep PSUM slots (4 slots/bank, 3-bank rotating
    groups), bias added via a rank-8 matmul (lhsT=(8,128) bias table,
    rhs=(8,512) one-hot pattern).
  - recurrent contribution uses the merged weight W2 = Whh @ Whr so the
    recurrence runs on u = o*tanh(c) (128-dim) and the output projection
    h = Whr@u moves OFF the critical path (batched every 32 steps).
  - per step: 8 accumulate matmuls (4 gates x 2 dirs) -> sigmoid/tanh on
    ACT straight out of PSUM -> c update on DVE (packed [i|f]*[g|c]) ->
    tanh(c) -> u, which feeds the next step's matmuls.
  - backward direction shares every instruction with forward; its time
    reversal is handled with negative-stride access patterns on the gx
    matmul rhs and projection rhs.

All matmul operands are bf16 (weights preprocessed on host, fp32 PSUM
accumulation, fp32 elementwise), which measured ~3.5e-3 scale-relative
absmax against the fp32 reference.
"""

import numpy as np
import ml_dtypes

BF = ml_dtypes.bfloat16
SEQ = 512          # sequence length
BC = 16            # batch per core
NCORES = 8
EMBED = 64
HID = 128
PROJ = 64
GS = 12            # steps per PSUM group (3 banks x 4 slots)
CH = 32            # proj chunk: steps per output-projection matmul
PERM = [0, 1, 3, 2]  # reference gate order i,f,g,o -> slot order i,f,o,g

_cache = {}


# ---------------------------------------------------------------------------
# BIR post-fix: this container's walrus encodes at most one semaphore wait
# per TPB_CTRL (Drain/EventSemaphore) instruction; Tile's kernel-tail drain
# aggregates several. Split the extra waits onto fresh single-wait Drains.
# ---------------------------------------------------------------------------
def _split_multi_waits(nc, mybir, limit=1):
    n = [0]

    def fresh():
        n[0] += 1
        return f"I-waitsplit-{n[0]}"

    for fn in nc.m.functions:
        for blk in fn.blocks:
            out = []
            for ins in blk.instructions:
                si = getattr(ins, "sync_info", None)
                if si is not None and si.on_wait and len(si.on_wait) > limit:
                    waits = list(si.on_wait)
                    for w in waits[:-limit]:
                        out.append(mybir.InstDrain(
                            name=fresh(), engine=ins.engine, debug=ins.debug,
                            ins=[], outs=[],
                            sync_info=mybir.SyncInfo(on_wait=[w], on_update=[]),
                        ))
                    si.on_wait = waits[-limit:]
                out.append(ins)
            blk.instructions = out


# ---------------------------------------------------------------------------
# Host-side weight preprocessing
# ---------------------------------------------------------------------------
def _bf(x):
    return np.ascontiguousarray(np.asarray(x, dtype=np.float32).astype(BF))


def _perm_rows(m):
    return np.concatenate([m[g * HID:(g + 1) * HID] for g in PERM], axis=0)


def _pack_dir(p):
    Wih = np.asarray(p["Wih"], np.float32)
    Whh = np.asarray(p["Whh"], np.float32)
    bias = np.asarray(p["bih"], np.float32) + np.asarray(p["bhh"], np.float32)
    Whr = np.asarray(p["Whr"], np.float32)
    W2 = Whh @ Whr
    # sigmoid(x) = (tanh(x/2)+1)/2: prescale the i,f,o gate rows (slot
    # gates 0..2) by 0.5 so ONE tanh ACT op covers all four gates; the
    # (t+1)/2 affine is fused into the DVE multiplies downstream.
    scl = np.ones((512, 1), np.float32)
    scl[:384] = 0.5
    # state conventions: u is stored as u' = 2u = (tanh(o/2)+1)*tanh(c),
    # so W2 and Whr absorb an extra 0.5 on their u-columns.
    Wp, W2p = _perm_rows(Wih) * scl, _perm_rows(W2) * scl * 0.5
    bp = _perm_rows(bias[:, None])[:, 0] * scl[:, 0]
    return {
        "WihT": Wp.T,                         # (in, 512)
        "bias": bp,                           # (512,)
        "W2T": W2p.T,                         # (128, 512)
        "WhrT": Whr.T * 0.5,                  # (128, 64)
    }


def _host_pack(embedding, params):
    """Build all replicated weight arrays (bf16) once."""
    out = {}
    for l in range(2):
        pk = {d: _pack_dir(params[l][d]) for d in ("fwd", "bwd")}
        out[f"wih{l}"] = _bf(np.concatenate(
            [pk["fwd"]["WihT"], pk["bwd"]["WihT"]], axis=1))      # (in, 1024)
        out[f"w2{l}"] = _bf(np.concatenate(
            [pk["fwd"]["W2T"], pk["bwd"]["W2T"]], axis=1))        # (128, 1024)
        out[f"whr{l}"] = _bf(np.concatenate(
            [pk["fwd"]["WhrT"], pk["bwd"]["WhrT"]], axis=1))      # (128, 128)
        # rank-8 bias table: row j=(g*2+d) -> bias_d[g*128 + p]
        b8 = np.zeros((8, HID), np.float32)
        for g in range(4):
            for d, dn in enumerate(("fwd", "bwd")):
                b8[g * 2 + d] = pk[dn]["bias"][g * HID:(g + 1) * HID]
        out[f"b8{l}"] = _bf(b8)
    # one-hot pattern (8, 512): col n = slot(4)*128 + g*32 + d*16 + b
    e8 = np.zeros((8, 512), np.float32)
    for sl in range(4):
        for g in range(4):
            for d in range(2):
                e8[g * 2 + d, sl * 128 + g * 32 + d * 16:
                   sl * 128 + g * 32 + (d + 1) * 16] = 1.0
    out["e8"] = _bf(e8)
    return out


# ---------------------------------------------------------------------------
# Device program
# ---------------------------------------------------------------------------
def _build(seq=SEQ):
    import concourse.bass as bass
    import concourse.mybir as mybir
    from concourse.tile import TileContext

    f32, bf16 = mybir.dt.float32, mybir.dt.bfloat16
    SIG = mybir.ActivationFunctionType.Sigmoid
    TANH = mybir.ActivationFunctionType.Tanh
    TOK = seq * BC

    nc = bass.Bass()
    dr_x0 = nc.dram_tensor("x0", [EMBED, TOK], bf16, kind="ExternalInput")
    dr_w = {}
    for l, kin in ((0, EMBED), (1, HID)):
        dr_w[f"wih{l}"] = nc.dram_tensor(f"wih{l}", [kin, 1024], bf16, kind="ExternalInput")
        dr_w[f"w2{l}"] = nc.dram_tensor(f"w2{l}", [128, 1024], bf16, kind="ExternalInput")
        dr_w[f"whr{l}"] = nc.dram_tensor(f"whr{l}", [128, 128], bf16, kind="ExternalInput")
        dr_w[f"b8{l}"] = nc.dram_tensor(f"b8{l}", [8, 128], bf16, kind="ExternalInput")
    dr_w["e8"] = nc.dram_tensor("e8", [8, 512], bf16, kind="ExternalInput")
    dr_out = nc.dram_tensor("out", [HID, TOK], f32, kind="ExternalOutput")

    n_groups = (seq + GS - 1) // GS
    n_chunks = seq // CH

    with TileContext(nc) as tc:
        with tc.tile_pool(name="const", bufs=1) as cp, \
             tc.tile_pool(name="state", bufs=1) as sp, \
             tc.tile_pool(name="act", bufs=4) as ap_pool, \
             tc.tile_pool(name="small", bufs=3) as smp, \
             tc.tile_pool(name="gx", bufs=2, space="PSUM") as gxp, \
             tc.tile_pool(name="proj", bufs=2, space="PSUM") as prp:

            # ---- load constants -------------------------------------------
            x0 = cp.tile([EMBED, TOK], bf16, tag="x0", name="x0t")
            nc.sync.dma_start(out=x0[:, :], in_=dr_x0[:, :])
            w = {}
            for l, kin in ((0, EMBED), (1, HID)):
                w[f"wih{l}"] = cp.tile([kin, 1024], bf16, tag=f"wih{l}", name=f"wih{l}")
                w[f"w2{l}"] = cp.tile([128, 1024], bf16, tag=f"w2{l}", name=f"w2{l}")
                w[f"whr{l}"] = cp.tile([128, 128], bf16, tag=f"whr{l}", name=f"whr{l}")
                w[f"b8{l}"] = cp.tile([8, 128], bf16, tag=f"b8{l}", name=f"b8{l}")
            w["e8"] = cp.tile([8, 512], bf16, tag="e8", name="e8")
            for k, t in w.items():
                nc.sync.dma_start(out=t[:, :], in_=dr_w[k][:, :])

            # warm the sigmoid/tanh activation table set early
            warm = cp.tile([128, 8], f32, tag="warm", name="warm")
            nc.gpsimd.memset(warm[:, :], 0.0)
            nc.scalar.activation(warm[:, 0:8], warm[:, 0:8], TANH)


            H0 = sp.tile([HID, TOK], bf16, tag="H0", name="H0")
            H1 = sp.tile([HID, TOK], f32, tag="H1", name="H1")

            for l in range(2):
                X = x0 if l == 0 else H0
                H = H0 if l == 0 else H1
                wih, w2t = w[f"wih{l}"], w[f"w2{l}"]
                whr, b8 = w[f"whr{l}"], w[f"b8{l}"]
                e8 = w["e8"]
                X3 = X.rearrange("p (t b) -> p t b", b=BC)

                U = sp.tile([HID, seq * 2 * BC], bf16, tag="U", name="U")
                U3 = U.rearrange("p (s u) -> p s u", u=2 * BC)

                # gx matmul emitters: group -> list of thunks (banked)
                def gx_mms(k, gt):
                    s_base = k * GS
                    gsteps = min(GS, seq - s_base)
                    nbank = gsteps // 4
                    thunks = []
                    for bank in range(nbank):
                        s0 = s_base + bank * 4
                        gtb = gt[:, bank * 512:(bank + 1) * 512].rearrange(
                            "p (sl c) -> p sl c", c=128)

                        def mk(d, g, s0=s0, gtb=gtb, bank=bank, first=False):
                            def run():
                                o = gtb[:, :, g * 32 + d * 16:g * 32 + d * 16 + 16]
                                if d == 0:
                                    rhs = X3[:, s0:s0 + 4, :]
                                else:
                                    hi = seq - 1 - s0
                                    lo = hi - 4
                                    rhs = (X3[:, hi::-1, :] if lo < 0
                                           else X3[:, hi:lo:-1, :])
                                nc.tensor.matmul(
                                    o, wih[:, d * 512 + g * 128:d * 512 + (g + 1) * 128],
                                    rhs, start=first, stop=False,
                                    skip_group_check=True)
                            return run

                        for idx, (d, g) in enumerate(
                                [(d, g) for d in (0, 1) for g in range(4)]):
                            thunks.append(mk(d, g, first=(idx == 0)))

                        def bias_mm(gt=gt, bank=bank):
                            nc.tensor.matmul(
                                gt[:, bank * 512:(bank + 1) * 512],
                                b8[:, :], e8[:, :],
                                start=False, stop=False, skip_group_check=True)
                        thunks.append(bias_mm)
                    return thunks

                # output projection chunks: (dir, chunk) ready at step
                proj_ready = {}
                dma_ready = {}
                post_proj = []
                post_dma = []
                for c in range(n_chunks):
                    s_f = c * CH + CH - 1 + 2   # +2: keep proj MMs out of the
                    s_b = seq - 1 - c * CH + 2  # recurrence-critical PE window
                    if s_f < seq:
                        proj_ready.setdefault(s_f, []).append((0, c))
                    else:
                        post_proj.append((0, c))
                    if s_b < seq:
                        proj_ready.setdefault(s_b, []).append((1, c))
                    else:
                        post_proj.append((1, c))
                    if l == 1:
                        s_d = max(s_f, s_b) + 1
                        if s_d < seq:
                            dma_ready.setdefault(s_d, []).append(c)
                        else:
                            post_dma.append(c)

                def emit_proj(d, c):
                    pt = prp.tile([PROJ, CH * BC], f32, tag="proj", name="proj")
                    if d == 0:
                        rhs = U3[:, c * CH:(c + 1) * CH, 0:BC]
                    else:
                        hi = seq - 1 - c * CH
                        lo = hi - CH
                        rhs = (U3[:, hi::-1, BC:2 * BC] if lo < 0
                               else U3[:, hi:lo:-1, BC:2 * BC])
                    nc.tensor.matmul(pt[:, :], whr[:, d * PROJ:(d + 1) * PROJ],
                                     rhs, start=True, stop=True,
                                     skip_group_check=True)
                    nc.scalar.copy(
                        H[d * PROJ:(d + 1) * PROJ, c * CH * BC:(c + 1) * CH * BC],
                        pt[:, :])

                # ---- the time loop ------------------------------------
                act_cur = ap_pool.tile([128, 160], f32, tag="act", name="act")
                nc.gpsimd.memset(act_cur[:, 128:160], 0.0)

                gt_tiles = {}
                gt_tiles[0] = gxp.tile([128, 512 * ((min(GS, seq) // 4))], f32, tag="gx", name="gx")
                for th in gx_mms(0, gt_tiles[0]):
                    th()
                pending = []   # thunks for next group, drained a few per step

                for s in range(seq):
                    k, pos = divmod(s, GS)
                    if pos == 0:
                        for th in pending:  # flush any leftover gx fills
                            th()
                        pending = []
                        # queue next group's gx fill, spread over early steps
                        if k + 1 < n_groups:
                            nb = (min(GS, seq - (k + 1) * GS)) // 4
                            gt_tiles[k + 1] = gxp.tile([128, 512 * nb], f32, tag="gx", name="gx")
                            pending = gx_mms(k + 1, gt_tiles[k + 1])
                        if k - 1 in gt_tiles:
                            del gt_tiles[k - 1]
                    gt = gt_tiles[k]
                    slot = gt[:, pos * 128:(pos + 1) * 128]

                    # recurrent matmuls accumulate onto gx+bias in PSUM
                    if s > 0:
                        for d in (0, 1):
                            for g in range(4):
                                nc.tensor.matmul(
                                    slot[:, g * 32 + d * 16:g * 32 + d * 16 + 16],
                                    w2t[:, d * 512 + g * 128:d * 512 + (g + 1) * 128],
                                    U[:, (s - 1) * 32 + d * 16:(s - 1) * 32 + d * 16 + 16],
                                    start=False, stop=(d == 1 and g == 3),
                                    skip_group_check=True)
                    # spread next group's input matmuls into chain slack
                    busy_proj = bool(proj_ready.get(s))
                    ndrain = 0 if busy_proj else (3 if pos < 9 else 2)
                    for _ in range(min(ndrain, len(pending))):
                        pending.pop(0)()

                    act_next = ap_pool.tile([128, 160], f32, tag="act", name="act")
                    # all four gates in ONE tanh (i,f,o preacts prescaled 0.5)
                    nc.scalar.activation(act_cur[:, 0:128], slot[:, 0:128], TANH)
                    tmp = smp.tile([128, 64], f32, tag="tmp", name="tmp")
                    # tmp = ([ti|tf]+1)*[g|c~] = [2*i*g | 4*f*c]
                    nc.vector.scalar_tensor_tensor(
                        tmp[:, :], act_cur[:, 0:64], 1.0, act_cur[:, 96:160],
                        mybir.AluOpType.add, mybir.AluOpType.mult)
                    # c~' = 2c' = 0.5*(4fc) + 2ig
                    nc.vector.scalar_tensor_tensor(
                        act_next[:, 128:160], tmp[:, 32:64], 0.5, tmp[:, 0:32],
                        mybir.AluOpType.mult, mybir.AluOpType.add)
                    tch = smp.tile([128, 32], f32, tag="tch", name="tch")
                    nc.scalar.activation(tch[:, :], act_next[:, 128:160], TANH,
                                         scale=0.5)
                    # u' = (to+1)*tanh(c)  (bf16, feeds next matmuls)
                    nc.vector.scalar_tensor_tensor(
                        U[:, s * 32:(s + 1) * 32], act_cur[:, 64:96], 1.0,
                        tch[:, :],
                        mybir.AluOpType.add, mybir.AluOpType.mult)
                    act_cur = act_next

                    for (d, c) in proj_ready.get(s, ()):
                        emit_proj(d, c)
                    if l == 1:
                        for c in dma_ready.get(s, ()):
                            nc.sync.dma_start(
                                out=dr_out[:, c * CH * BC:(c + 1) * CH * BC],
                                in_=H1[:, c * CH * BC:(c + 1) * CH * BC])
                for (d, c) in post_proj:
                    emit_proj(d, c)
                if l == 1:
                    for c in post_dma:
                        nc.sync.dma_start(
                            out=dr_out[:, c * CH * BC:(c + 1) * CH * BC],
                            in_=H1[:, c * CH * BC:(c + 1) * CH * BC])

    _split_multi_waits(nc, mybir)
    return nc


def _get_nc(seq=SEQ):
    if seq not in _cache:
        _cache[seq] = _build(seq)
    return _cache[seq]


# ---------------------------------------------------------------------------
# Public entry point
# ---------------------------------------------------------------------------
def kernel(x, embedding, params):
    from concourse.bass_utils import run_bass_kernel_spmd

    x = np.asarray(x)
    emb = np.asarray(embedding, np.float32)
    wpack = _host_pack(emb, params)

    emb_x = emb[x]                     # (512, 128, 64)
    in_maps = []
    for c in range(NCORES):
        xc = emb_x[:, c * BC:(c + 1) * BC, :]          # (512, 16, 64)
        x_fm = _bf(xc.transpose(2, 0, 1).reshape(EMBED, SEQ * BC))
        m = {"x0": x_fm}
        m.update(wpack)
        in_maps.append(m)

    nc = _get_nc(SEQ)
    res = run_bass_kernel_spmd(nc, in_maps, core_ids=list(range(NCORES)))

    outs = []
    for c in range(NCORES):
        H1 = res.results[c]["out"]                     # (128, 8192) f32
        outs.append(H1.reshape(HID, SEQ, BC).transpose(1, 2, 0))
    return np.concatenate(outs, axis=1).astype(np.float32)  # (512, 128, 128)


# revision 11
# speedup vs baseline: 1.0444x; 1.0444x over previous
"""Trainium2 Bass kernel for a 2-layer bidirectional projected LSTM encoder.

Problem: x (512, 128) int32 tokens -> embedding (30, 64) -> 2 layers of
bidirectional LSTM (hidden 128, proj 64) -> output (512, 128, 128) f32.

Strategy (per spec sharding hint): data-parallel over batch, 16 batch
elements per NeuronCore, weights replicated. Per core, the two directions
of a layer are fused into shared instructions (feature-major layout, 128
partitions = hidden unit, free dim = [gate|dir|batch]).

Key device-side structure per layer:
  - "gx" input contributions Wih@x + bias are computed by batched matmuls
    directly into per-timestep PSUM slots (4 slots/bank, 3-bank rotating
    groups), bias added via a rank-8 matmul (lhsT=(8,128) bias table,
    rhs=(8,512) one-hot pattern).
  - recurrent contribution uses the merged weight W2 = Whh @ Whr so the
    recurrence runs on u = o*tanh(c) (128-dim) and the output projection
    h = Whr@u moves OFF the critical path (batched every 32 steps).
  - per step: 8 accumulate matmuls (4 gates x 2 dirs) -> sigmoid/tanh on
    ACT straight out of PSUM -> c update on DVE (packed [i|f]*[g|c]) ->
    tanh(c) -> u, which feeds the next step's matmuls.
  - backward direction shares every instruction with forward; its time
    reversal is handled with negative-stride access patterns on the gx
    matmul rhs and projection rhs.

All matmul operands are bf16 (weights preprocessed on host, fp32 PSUM
accumulation, fp32 elementwise), which measured ~3.5e-3 scale-relative
absmax against the fp32 reference.
"""

import numpy as np
import ml_dtypes

BF = ml_dtypes.bfloat16
SEQ = 512          # sequence length
BC = 16            # batch per core
NCORES = 8
EMBED = 64
HID = 128
PROJ = 64
GS = 12            # steps per PSUM group (3 banks x 4 slots)
CH = 32            # proj chunk: steps per output-projection matmul
PERM = [0, 1, 3, 2]  # reference gate order i,f,g,o -> slot order i,f,o,g

_cache = {}


# ---------------------------------------------------------------------------
# BIR post-fix: this container's walrus encodes at most one semaphore wait
# per TPB_CTRL (Drain/EventSemaphore) instruction; Tile's kernel-tail drain
# aggregates several. Split the extra waits onto fresh single-wait Drains.
# ---------------------------------------------------------------------------
def _split_multi_waits(nc, mybir, limit=1):
    n = [0]

    def fresh():
        n[0] += 1
        return f"I-waitsplit-{n[0]}"

    for fn in nc.m.functions:
        for blk in fn.blocks:
            out = []
            for ins in blk.instructions:
                si = getattr(ins, "sync_info", None)
                if si is not None and si.on_wait and len(si.on_wait) > limit:
                    waits = list(si.on_wait)
                    for w in waits[:-limit]:
                        out.append(mybir.InstDrain(
                            name=fresh(), engine=ins.engine, debug=ins.debug,
                            ins=[], outs=[],
                            sync_info=mybir.SyncInfo(on_wait=[w], on_update=[]),
                        ))
                    si.on_wait = waits[-limit:]
                out.append(ins)
            blk.instructions = out


# ---------------------------------------------------------------------------
# Host-side weight preprocessing
# ---------------------------------------------------------------------------
def _bf(x):
    return np.ascontiguousarray(np.asarray(x, dtype=np.float32).astype(BF))


def _perm_rows(m):
    return np.concatenate([m[g * HID:(g + 1) * HID] for g in PERM], axis=0)


def _pack_dir(p):
    Wih = np.asarray(p["Wih"], np.float32)
    Whh = np.asarray(p["Whh"], np.float32)
    bias = np.asarray(p["bih"], np.float32) + np.asarray(p["bhh"], np.float32)
    Whr = np.asarray(p["Whr"], np.float32)
    W2 = Whh @ Whr
    # sigmoid(x) = (tanh(x/2)+1)/2: prescale the i,f,o gate rows (slot
    # gates 0..2) by 0.5 so ONE tanh ACT op covers all four gates; the
    # (t+1)/2 affine is fused into the DVE multiplies downstream.
    scl = np.ones((512, 1), np.float32)
    scl[:384] = 0.5
    # state conventions: u is stored as u' = 2u = (tanh(o/2)+1)*tanh(c),
    # so W2 and Whr absorb an extra 0.5 on their u-columns.
    Wp, W2p = _perm_rows(Wih) * scl, _perm_rows(W2) * scl * 0.5
    bp = _perm_rows(bias[:, None])[:, 0] * scl[:, 0]
    return {
        "WihT": Wp.T,                         # (in, 512)
        "bias": bp,                           # (512,)
        "W2T": W2p.T,                         # (128, 512)
        "WhrT": Whr.T * 0.5,                  # (128, 64)
    }


def _host_pack(embedding, params):
    """Build all replicated weight arrays (bf16) once."""
    out = {}
    for l in range(2):
        pk = {d: _pack_dir(params[l][d]) for d in ("fwd", "bwd")}
        out[f"wih{l}"] = _bf(np.concatenate(
            [pk["fwd"]["WihT"], pk["bwd"]["WihT"]], axis=1))      # (in, 1024)
        out[f"w2{l}"] = _bf(np.concatenate(
            [pk["fwd"]["W2T"], pk["bwd"]["W2T"]], axis=1))        # (128, 1024)
        out[f"whr{l}"] = _bf(np.concatenate(
            [pk["fwd"]["WhrT"], pk["bwd"]["WhrT"]], axis=1))      # (128, 128)
        # rank-8 bias table: row j=(g*2+d) -> bias_d[g*128 + p]
        b8 = np.zeros((8, HID), np.float32)
        for g in range(4):
            for d, dn in enumerate(("fwd", "bwd")):
                b8[g * 2 + d] = pk[dn]["bias"][g * HID:(g + 1) * HID]
        out[f"b8{l}"] = _bf(b8)
    # one-hot pattern (8, 512): col n = slot(4)*128 + g*32 + d*16 + b
    e8 = np.zeros((8, 512), np.float32)
    for sl in range(4):
        for g in range(4):
            for d in range(2):
                e8[g * 2 + d, sl * 128 + g * 32 + d * 16:
                   sl * 128 + g * 32 + (d + 1) * 16] = 1.0
    out["e8"] = _bf(e8)
    return out


# ---------------------------------------------------------------------------
# Device program
# ---------------------------------------------------------------------------
def _build(seq=SEQ):
    import concourse.bass as bass
    import concourse.mybir as mybir
    from concourse.tile import TileContext

    f32, bf16 = mybir.dt.float32, mybir.dt.bfloat16
    SIG = mybir.ActivationFunctionType.Sigmoid
    TANH = mybir.ActivationFunctionType.Tanh
    TOK = seq * BC

    nc = bass.Bass()
    dr_x0 = nc.dram_tensor("x0", [EMBED, TOK], bf16, kind="ExternalInput")
    dr_w = {}
    for l, kin in ((0, EMBED), (1, HID)):
        dr_w[f"wih{l}"] = nc.dram_tensor(f"wih{l}", [kin, 1024], bf16, kind="ExternalInput")
        dr_w[f"w2{l}"] = nc.dram_tensor(f"w2{l}", [128, 1024], bf16, kind="ExternalInput")
        dr_w[f"whr{l}"] = nc.dram_tensor(f"whr{l}", [128, 128], bf16, kind="ExternalInput")
        dr_w[f"b8{l}"] = nc.dram_tensor(f"b8{l}", [8, 128], bf16, kind="ExternalInput")
    dr_w["e8"] = nc.dram_tensor("e8", [8, 512], bf16, kind="ExternalInput")
    dr_out = nc.dram_tensor("out", [HID, TOK], f32, kind="ExternalOutput")

    n_groups = (seq + GS - 1) // GS
    n_chunks = seq // CH

    with TileContext(nc) as tc:
        with tc.tile_pool(name="const", bufs=1) as cp, \
             tc.tile_pool(name="state", bufs=1) as sp, \
             tc.tile_pool(name="act", bufs=4) as ap_pool, \
             tc.tile_pool(name="small", bufs=3) as smp, \
             tc.tile_pool(name="gx", bufs=2, space="PSUM") as gxp, \
             tc.tile_pool(name="proj", bufs=2, space="PSUM") as prp:

            # ---- load constants -------------------------------------------
            x0 = cp.tile([EMBED, TOK], bf16, tag="x0", name="x0t")
            nc.sync.dma_start(out=x0[:, :], in_=dr_x0[:, :])
            w = {}
            for l, kin in ((0, EMBED), (1, HID)):
                w[f"wih{l}"] = cp.tile([kin, 1024], bf16, tag=f"wih{l}", name=f"wih{l}")
                w[f"w2{l}"] = cp.tile([128, 1024], bf16, tag=f"w2{l}", name=f"w2{l}")
                w[f"whr{l}"] = cp.tile([128, 128], bf16, tag=f"whr{l}", name=f"whr{l}")
                w[f"b8{l}"] = cp.tile([8, 128], bf16, tag=f"b8{l}", name=f"b8{l}")
            w["e8"] = cp.tile([8, 512], bf16, tag="e8", name="e8")
            for k, t in w.items():
                nc.sync.dma_start(out=t[:, :], in_=dr_w[k][:, :])

            # warm the sigmoid/tanh activation table set early
            warm = cp.tile([128, 8], f32, tag="warm", name="warm")
            nc.gpsimd.memset(warm[:, :], 0.0)
            nc.scalar.activation(warm[:, 0:8], warm[:, 0:8], TANH)


            H0 = sp.tile([HID, TOK], bf16, tag="H0", name="H0")
            H1 = sp.tile([HID, TOK], f32, tag="H1", name="H1")

            for l in range(2):
                X = x0 if l == 0 else H0
                H = H0 if l == 0 else H1
                wih, w2t = w[f"wih{l}"], w[f"w2{l}"]
                whr, b8 = w[f"whr{l}"], w[f"b8{l}"]
                e8 = w["e8"]
                X3 = X.rearrange("p (t b) -> p t b", b=BC)

                U = sp.tile([HID, seq * 2 * BC], bf16, tag="U", name="U")
                U3 = U.rearrange("p (s u) -> p s u", u=2 * BC)

                # gx matmul emitters: group -> list of thunks (banked)
                def gx_mms(k, gt):
                    s_base = k * GS
                    gsteps = min(GS, seq - s_base)
                    nbank = gsteps // 4
                    thunks = []
                    for bank in range(nbank):
                        s0 = s_base + bank * 4
                        gtb = gt[:, bank * 512:(bank + 1) * 512].rearrange(
                            "p (sl c) -> p sl c", c=128)

                        def mk(d, g, s0=s0, gtb=gtb, bank=bank, first=False):
                            def run():
                                o = gtb[:, :, g * 32 + d * 16:g * 32 + d * 16 + 16]
                                if d == 0:
                                    rhs = X3[:, s0:s0 + 4, :]
                                else:
                                    hi = seq - 1 - s0
                                    lo = hi - 4
                                    rhs = (X3[:, hi::-1, :] if lo < 0
                                           else X3[:, hi:lo:-1, :])
                                return nc.tensor.matmul(
                                    o, wih[:, d * 512 + g * 128:d * 512 + (g + 1) * 128],
                                    rhs, start=first, stop=False,
                                    skip_group_check=True)
                            return run

                        for idx, (d, g) in enumerate(
                                [(d, g) for d in (0, 1) for g in range(4)]):
                            thunks.append(mk(d, g, first=(idx == 0)))

                        def bias_mm(gt=gt, bank=bank):
                            return nc.tensor.matmul(
                                gt[:, bank * 512:(bank + 1) * 512],
                                b8[:, :], e8[:, :],
                                start=False, stop=False, skip_group_check=True)
                        thunks.append(bias_mm)
                    return thunks

                # output projection chunks: (dir, chunk) ready at step
                proj_ready = {}
                dma_ready = {}
                post_proj = []
                post_dma = []
                for c in range(n_chunks):
                    s_f = c * CH + CH - 1 + 2   # +2: keep proj MMs out of the
                    s_b = seq - 1 - c * CH + 2  # recurrence-critical PE window
                    if s_f < seq:
                        proj_ready.setdefault(s_f, []).append((0, c))
                    else:
                        post_proj.append((0, c))
                    if s_b < seq:
                        proj_ready.setdefault(s_b, []).append((1, c))
                    else:
                        post_proj.append((1, c))
                    if l == 1:
                        s_d = max(s_f, s_b) + 1
                        if s_d < seq:
                            dma_ready.setdefault(s_d, []).append(c)
                        else:
                            post_dma.append(c)

                def emit_proj(d, c):
                    pt = prp.tile([PROJ, CH * BC], f32, tag="proj", name="proj")
                    if d == 0:
                        rhs = U3[:, c * CH:(c + 1) * CH, 0:BC]
                    else:
                        hi = seq - 1 - c * CH
                        lo = hi - CH
                        rhs = (U3[:, hi::-1, BC:2 * BC] if lo < 0
                               else U3[:, hi:lo:-1, BC:2 * BC])
                    nc.tensor.matmul(pt[:, :], whr[:, d * PROJ:(d + 1) * PROJ],
                                     rhs, start=True, stop=True,
                                     skip_group_check=True)
                    nc.scalar.copy(
                        H[d * PROJ:(d + 1) * PROJ, c * CH * BC:(c + 1) * CH * BC],
                        pt[:, :])

                # ---- the time loop ------------------------------------
                act_cur = ap_pool.tile([128, 160], f32, tag="act", name="act")
                nc.gpsimd.memset(act_cur[:, 128:160], 0.0)

                gt_tiles = {}
                gt_tiles[0] = gxp.tile([128, 512 * ((min(GS, seq) // 4))], f32, tag="gx", name="gx")
                for th in gx_mms(0, gt_tiles[0]):
                    th()
                pending = []   # thunks for next group, drained a few per step

                for s in range(seq):
                    k, pos = divmod(s, GS)
                    if pos == 0:
                        for th in pending:  # flush any leftover gx fills
                            th()
                        pending = []
                        # queue next group's gx fill, spread over early steps
                        if k + 1 < n_groups:
                            nb = (min(GS, seq - (k + 1) * GS)) // 4
                            gt_tiles[k + 1] = gxp.tile([128, 512 * nb], f32, tag="gx", name="gx")
                            pending = gx_mms(k + 1, gt_tiles[k + 1])
                        if k - 1 in gt_tiles:
                            del gt_tiles[k - 1]
                    gt = gt_tiles[k]
                    slot = gt[:, pos * 128:(pos + 1) * 128]

                    # recurrent matmuls accumulate onto gx+bias in PSUM
                    anchor = None
                    if s > 0:
                        for d in (0, 1):
                            for g in range(4):
                                anchor = nc.tensor.matmul(
                                    slot[:, g * 32 + d * 16:g * 32 + d * 16 + 16],
                                    w2t[:, d * 512 + g * 128:d * 512 + (g + 1) * 128],
                                    U[:, (s - 1) * 32 + d * 16:(s - 1) * 32 + d * 16 + 16],
                                    start=False, stop=(d == 1 and g == 3),
                                    skip_group_check=True)
                    # spread next group's input matmuls into chain slack
                    busy_proj = bool(proj_ready.get(s))
                    ndrain = 0 if busy_proj else (3 if pos < 9 else 2)
                    for _ in range(min(ndrain, len(pending))):
                        inst = pending.pop(0)()
                        if anchor is not None and inst is not None:
                            from concourse.tile import add_dep_helper
                            add_dep_helper(inst.ins, anchor.ins, sync=False,
                                           reason="spread gx fills")

                    act_next = ap_pool.tile([128, 160], f32, tag="act", name="act")
                    # all four gates in ONE tanh (i,f,o preacts prescaled 0.5)
                    nc.scalar.activation(act_cur[:, 0:128], slot[:, 0:128], TANH)
                    tmp = smp.tile([128, 64], f32, tag="tmp", name="tmp")
                    # tmp = ([ti|tf]+1)*[g|c~] = [2*i*g | 4*f*c]
                    nc.vector.scalar_tensor_tensor(
                        tmp[:, :], act_cur[:, 0:64], 1.0, act_cur[:, 96:160],
                        mybir.AluOpType.add, mybir.AluOpType.mult)
                    # c~' = 2c' = 0.5*(4fc) + 2ig
                    nc.vector.scalar_tensor_tensor(
                        act_next[:, 128:160], tmp[:, 32:64], 0.5, tmp[:, 0:32],
                        mybir.AluOpType.mult, mybir.AluOpType.add)
                    tch = smp.tile([128, 32], f32, tag="tch", name="tch")
                    nc.scalar.activation(tch[:, :], act_next[:, 128:160], TANH,
                                         scale=0.5)
                    # u' = (to+1)*tanh(c)  (bf16, feeds next matmuls)
                    nc.vector.scalar_tensor_tensor(
                        U[:, s * 32:(s + 1) * 32], act_cur[:, 64:96], 1.0,
                        tch[:, :],
                        mybir.AluOpType.add, mybir.AluOpType.mult)
                    act_cur = act_next

                    for (d, c) in proj_ready.get(s, ()):
                        emit_proj(d, c)
                    if l == 1:
                        for c in dma_ready.get(s, ()):
                            nc.sync.dma_start(
                                out=dr_out[:, c * CH * BC:(c + 1) * CH * BC],
                                in_=H1[:, c * CH * BC:(c + 1) * CH * BC])
                for (d, c) in post_proj:
                    emit_proj(d, c)
                if l == 1:
                    for c in post_dma:
                        nc.sync.dma_start(
                            out=dr_out[:, c * CH * BC:(c + 1) * CH * BC],
                            in_=H1[:, c * CH * BC:(c + 1) * CH * BC])

    _split_multi_waits(nc, mybir)
    return nc


def _get_nc(seq=SEQ):
    if seq not in _cache:
        _cache[seq] = _build(seq)
    return _cache[seq]


# ---------------------------------------------------------------------------
# Public entry point
# ---------------------------------------------------------------------------
def kernel(x, embedding, params):
    from concourse.bass_utils import run_bass_kernel_spmd

    x = np.asarray(x)
    emb = np.asarray(embedding, np.float32)
    wpack = _host_pack(emb, params)

    emb_x = emb[x]                     # (512, 128, 64)
    in_maps = []
    for c in range(NCORES):
        xc = emb_x[:, c * BC:(c + 1) * BC, :]          # (512, 16, 64)
        x_fm = _bf(xc.transpose(2, 0, 1).reshape(EMBED, SEQ * BC))
        m = {"x0": x_fm}
        m.update(wpack)
        in_maps.append(m)

    nc = _get_nc(SEQ)
    res = run_bass_kernel_spmd(nc, in_maps, core_ids=list(range(NCORES)))

    outs = []
    for c in range(NCORES):
        H1 = res.results[c]["out"]                     # (128, 8192) f32
        outs.append(H1.reshape(HID, SEQ, BC).transpose(1, 2, 0))
    return np.concatenate(outs, axis=1).astype(np.float32)  # (512, 128, 128)


# revision 12
# speedup vs baseline: 1.0491x; 1.0046x over previous
"""Trainium2 Bass kernel for a 2-layer bidirectional projected LSTM encoder.

Problem: x (512, 128) int32 tokens -> embedding (30, 64) -> 2 layers of
bidirectional LSTM (hidden 128, proj 64) -> output (512, 128, 128) f32.

Strategy (per spec sharding hint): data-parallel over batch, 16 batch
elements per NeuronCore, weights replicated. Per core, the two directions
of a layer are fused into shared instructions (feature-major layout, 128
partitions = hidden unit, free dim = [gate|dir|batch]).

Key device-side structure per layer:
  - "gx" input contributions Wih@x + bias are computed by batched matmuls
    directly into per-timestep PSUM slots (4 slots/bank, 3-bank rotating
    groups), bias added via a rank-8 matmul (lhsT=(8,128) bias table,
    rhs=(8,512) one-hot pattern).
  - recurrent contribution uses the merged weight W2 = Whh @ Whr so the
    recurrence runs on u = o*tanh(c) (128-dim) and the output projection
    h = Whr@u moves OFF the critical path (batched every 32 steps).
  - per step: 8 accumulate matmuls (4 gates x 2 dirs) -> sigmoid/tanh on
    ACT straight out of PSUM -> c update on DVE (packed [i|f]*[g|c]) ->
    tanh(c) -> u, which feeds the next step's matmuls.
  - backward direction shares every instruction with forward; its time
    reversal is handled with negative-stride access patterns on the gx
    matmul rhs and projection rhs.

All matmul operands are bf16 (weights preprocessed on host, fp32 PSUM
accumulation, fp32 elementwise), which measured ~3.5e-3 scale-relative
absmax against the fp32 reference.
"""

import numpy as np
import ml_dtypes

BF = ml_dtypes.bfloat16
SEQ = 512          # sequence length
BC = 16            # batch per core
NCORES = 8
EMBED = 64
HID = 128
PROJ = 64
GS = 12            # steps per PSUM group (3 banks x 4 slots)
CH = 32            # proj chunk: steps per output-projection matmul
PERM = [0, 1, 3, 2]  # reference gate order i,f,g,o -> slot order i,f,o,g

_cache = {}


# ---------------------------------------------------------------------------
# BIR post-fix: this container's walrus encodes at most one semaphore wait
# per TPB_CTRL (Drain/EventSemaphore) instruction; Tile's kernel-tail drain
# aggregates several. Split the extra waits onto fresh single-wait Drains.
# ---------------------------------------------------------------------------
def _split_multi_waits(nc, mybir, limit=1):
    n = [0]

    def fresh():
        n[0] += 1
        return f"I-waitsplit-{n[0]}"

    for fn in nc.m.functions:
        for blk in fn.blocks:
            out = []
            for ins in blk.instructions:
                si = getattr(ins, "sync_info", None)
                if si is not None and si.on_wait and len(si.on_wait) > limit:
                    waits = list(si.on_wait)
                    for w in waits[limit:]:
                        out.append(mybir.InstDrain(
                            name=fresh(), engine=ins.engine, debug=ins.debug,
                            ins=[], outs=[],
                            sync_info=mybir.SyncInfo(on_wait=[w], on_update=[]),
                        ))
                    si.on_wait = waits[:limit]
                out.append(ins)
            blk.instructions = out


# ---------------------------------------------------------------------------
# Host-side weight preprocessing
# ---------------------------------------------------------------------------
def _bf(x):
    return np.ascontiguousarray(np.asarray(x, dtype=np.float32).astype(BF))


def _perm_rows(m):
    return np.concatenate([m[g * HID:(g + 1) * HID] for g in PERM], axis=0)


def _pack_dir(p):
    Wih = np.asarray(p["Wih"], np.float32)
    Whh = np.asarray(p["Whh"], np.float32)
    bias = np.asarray(p["bih"], np.float32) + np.asarray(p["bhh"], np.float32)
    Whr = np.asarray(p["Whr"], np.float32)
    W2 = Whh @ Whr
    # sigmoid(x) = (tanh(x/2)+1)/2: prescale the i,f,o gate rows (slot
    # gates 0..2) by 0.5 so ONE tanh ACT op covers all four gates; the
    # (t+1)/2 affine is fused into the DVE multiplies downstream.
    scl = np.ones((512, 1), np.float32)
    scl[:384] = 0.5
    # state conventions: u is stored as u' = 2u = (tanh(o/2)+1)*tanh(c),
    # so W2 and Whr absorb an extra 0.5 on their u-columns.
    Wp, W2p = _perm_rows(Wih) * scl, _perm_rows(W2) * scl * 0.5
    bp = _perm_rows(bias[:, None])[:, 0] * scl[:, 0]
    return {
        "WihT": Wp.T,                         # (in, 512)
        "bias": bp,                           # (512,)
        "W2T": W2p.T,                         # (128, 512)
        "WhrT": Whr.T * 0.5,                  # (128, 64)
    }


def _host_pack(embedding, params):
    """Build all replicated weight arrays (bf16) once."""
    out = {}
    for l in range(2):
        pk = {d: _pack_dir(params[l][d]) for d in ("fwd", "bwd")}
        out[f"wih{l}"] = _bf(np.concatenate(
            [pk["fwd"]["WihT"], pk["bwd"]["WihT"]], axis=1))      # (in, 1024)
        out[f"w2{l}"] = _bf(np.concatenate(
            [pk["fwd"]["W2T"], pk["bwd"]["W2T"]], axis=1))        # (128, 1024)
        out[f"whr{l}"] = _bf(np.concatenate(
            [pk["fwd"]["WhrT"], pk["bwd"]["WhrT"]], axis=1))      # (128, 128)
        # rank-8 bias table: row j=(g*2+d) -> bias_d[g*128 + p]
        b8 = np.zeros((8, HID), np.float32)
        for g in range(4):
            for d, dn in enumerate(("fwd", "bwd")):
                b8[g * 2 + d] = pk[dn]["bias"][g * HID:(g + 1) * HID]
        out[f"b8{l}"] = _bf(b8)
    # one-hot pattern (8, 512): col n = slot(4)*128 + g*32 + d*16 + b
    e8 = np.zeros((8, 512), np.float32)
    for sl in range(4):
        for g in range(4):
            for d in range(2):
                e8[g * 2 + d, sl * 128 + g * 32 + d * 16:
                   sl * 128 + g * 32 + (d + 1) * 16] = 1.0
    out["e8"] = _bf(e8)
    return out


# ---------------------------------------------------------------------------
# Device program
# ---------------------------------------------------------------------------
def _build(seq=SEQ):
    import concourse.bass as bass
    import concourse.mybir as mybir
    from concourse.tile import TileContext

    f32, bf16 = mybir.dt.float32, mybir.dt.bfloat16
    SIG = mybir.ActivationFunctionType.Sigmoid
    TANH = mybir.ActivationFunctionType.Tanh
    TOK = seq * BC

    nc = bass.Bass()
    dr_x0 = nc.dram_tensor("x0", [EMBED, TOK], bf16, kind="ExternalInput")
    dr_w = {}
    for l, kin in ((0, EMBED), (1, HID)):
        dr_w[f"wih{l}"] = nc.dram_tensor(f"wih{l}", [kin, 1024], bf16, kind="ExternalInput")
        dr_w[f"w2{l}"] = nc.dram_tensor(f"w2{l}", [128, 1024], bf16, kind="ExternalInput")
        dr_w[f"whr{l}"] = nc.dram_tensor(f"whr{l}", [128, 128], bf16, kind="ExternalInput")
        dr_w[f"b8{l}"] = nc.dram_tensor(f"b8{l}", [8, 128], bf16, kind="ExternalInput")
    dr_w["e8"] = nc.dram_tensor("e8", [8, 512], bf16, kind="ExternalInput")
    dr_out = nc.dram_tensor("out", [HID, TOK], f32, kind="ExternalOutput")

    n_groups = (seq + GS - 1) // GS
    n_chunks = seq // CH

    with TileContext(nc) as tc:
        with tc.tile_pool(name="const", bufs=1) as cp, \
             tc.tile_pool(name="state", bufs=1) as sp, \
             tc.tile_pool(name="act", bufs=4) as ap_pool, \
             tc.tile_pool(name="small", bufs=3) as smp, \
             tc.tile_pool(name="gx", bufs=2, space="PSUM") as gxp, \
             tc.tile_pool(name="proj", bufs=2, space="PSUM") as prp:

            # ---- load constants -------------------------------------------
            x0 = cp.tile([EMBED, TOK], bf16, tag="x0", name="x0t")
            nc.sync.dma_start(out=x0[:, :], in_=dr_x0[:, :])
            w = {}
            for l, kin in ((0, EMBED), (1, HID)):
                w[f"wih{l}"] = cp.tile([kin, 1024], bf16, tag=f"wih{l}", name=f"wih{l}")
                w[f"w2{l}"] = cp.tile([128, 1024], bf16, tag=f"w2{l}", name=f"w2{l}")
                w[f"whr{l}"] = cp.tile([128, 128], bf16, tag=f"whr{l}", name=f"whr{l}")
                w[f"b8{l}"] = cp.tile([8, 128], bf16, tag=f"b8{l}", name=f"b8{l}")
            w["e8"] = cp.tile([8, 512], bf16, tag="e8", name="e8")
            for k, t in w.items():
                nc.sync.dma_start(out=t[:, :], in_=dr_w[k][:, :])

            # warm the sigmoid/tanh activation table set early
            warm = cp.tile([128, 8], f32, tag="warm", name="warm")
            nc.gpsimd.memset(warm[:, :], 0.0)
            nc.scalar.activation(warm[:, 0:8], warm[:, 0:8], TANH)


            H0 = sp.tile([HID, TOK], bf16, tag="H0", name="H0")
            H1 = sp.tile([HID, TOK], f32, tag="H1", name="H1")

            for l in range(2):
                X = x0 if l == 0 else H0
                H = H0 if l == 0 else H1
                wih, w2t = w[f"wih{l}"], w[f"w2{l}"]
                whr, b8 = w[f"whr{l}"], w[f"b8{l}"]
                e8 = w["e8"]
                X3 = X.rearrange("p (t b) -> p t b", b=BC)

                U = sp.tile([HID, seq * 2 * BC], bf16, tag="U", name="U")
                U3 = U.rearrange("p (s u) -> p s u", u=2 * BC)

                # gx matmul emitters: group -> list of thunks (banked)
                def gx_mms(k, gt):
                    s_base = k * GS
                    gsteps = min(GS, seq - s_base)
                    nbank = gsteps // 4
                    thunks = []
                    for bank in range(nbank):
                        s0 = s_base + bank * 4
                        gtb = gt[:, bank * 512:(bank + 1) * 512].rearrange(
                            "p (sl c) -> p sl c", c=128)

                        def mk(d, g, s0=s0, gtb=gtb, bank=bank, first=False):
                            def run():
                                o = gtb[:, :, g * 32 + d * 16:g * 32 + d * 16 + 16]
                                if d == 0:
                                    rhs = X3[:, s0:s0 + 4, :]
                                else:
                                    hi = seq - 1 - s0
                                    lo = hi - 4
                                    rhs = (X3[:, hi::-1, :] if lo < 0
                                           else X3[:, hi:lo:-1, :])
                                return nc.tensor.matmul(
                                    o, wih[:, d * 512 + g * 128:d * 512 + (g + 1) * 128],
                                    rhs, start=first, stop=False,
                                    skip_group_check=True)
                            return run

                        for idx, (d, g) in enumerate(
                                [(d, g) for d in (0, 1) for g in range(4)]):
                            thunks.append(mk(d, g, first=(idx == 0)))

                        def bias_mm(gt=gt, bank=bank):
                            return nc.tensor.matmul(
                                gt[:, bank * 512:(bank + 1) * 512],
                                b8[:, :], e8[:, :],
                                start=False, stop=False, skip_group_check=True)
                        thunks.append(bias_mm)
                    return thunks

                # output projection chunks: (dir, chunk) ready at step
                proj_ready = {}
                dma_ready = {}
                post_proj = []
                post_dma = []
                for c in range(n_chunks):
                    s_f = c * CH + CH - 1 + 2   # +2: keep proj MMs out of the
                    s_b = seq - 1 - c * CH + 2  # recurrence-critical PE window
                    if s_f < seq:
                        proj_ready.setdefault(s_f, []).append((0, c))
                    else:
                        post_proj.append((0, c))
                    if s_b < seq:
                        proj_ready.setdefault(s_b, []).append((1, c))
                    else:
                        post_proj.append((1, c))
                    if l == 1:
                        s_d = max(s_f, s_b) + 1
                        if s_d < seq:
                            dma_ready.setdefault(s_d, []).append(c)
                        else:
                            post_dma.append(c)

                def emit_proj(d, c):
                    pt = prp.tile([PROJ, CH * BC], f32, tag="proj", name="proj")
                    if d == 0:
                        rhs = U3[:, c * CH:(c + 1) * CH, 0:BC]
                    else:
                        hi = seq - 1 - c * CH
                        lo = hi - CH
                        rhs = (U3[:, hi::-1, BC:2 * BC] if lo < 0
                               else U3[:, hi:lo:-1, BC:2 * BC])
                    nc.tensor.matmul(pt[:, :], whr[:, d * PROJ:(d + 1) * PROJ],
                                     rhs, start=True, stop=True,
                                     skip_group_check=True)
                    nc.scalar.copy(
                        H[d * PROJ:(d + 1) * PROJ, c * CH * BC:(c + 1) * CH * BC],
                        pt[:, :])

                # ---- the time loop ------------------------------------
                act_cur = ap_pool.tile([128, 160], f32, tag="act", name="act")
                nc.gpsimd.memset(act_cur[:, 128:160], 0.0)

                gt_tiles = {}
                gt_tiles[0] = gxp.tile([128, 512 * ((min(GS, seq) // 4))], f32, tag="gx", name="gx")
                for th in gx_mms(0, gt_tiles[0]):
                    th()
                pending = []   # thunks for next group, drained a few per step

                for s in range(seq):
                    k, pos = divmod(s, GS)
                    if pos == 0:
                        for th in pending:  # flush any leftover gx fills
                            th()
                        pending = []
                        # queue next group's gx fill, spread over early steps
                        if k + 1 < n_groups:
                            nb = (min(GS, seq - (k + 1) * GS)) // 4
                            gt_tiles[k + 1] = gxp.tile([128, 512 * nb], f32, tag="gx", name="gx")
                            pending = gx_mms(k + 1, gt_tiles[k + 1])
                        if k - 1 in gt_tiles:
                            del gt_tiles[k - 1]
                    gt = gt_tiles[k]
                    slot = gt[:, pos * 128:(pos + 1) * 128]

                    # recurrent matmuls accumulate onto gx+bias in PSUM
                    anchor = None
                    if s > 0:
                        for d in (0, 1):
                            for g in range(4):
                                anchor = nc.tensor.matmul(
                                    slot[:, g * 32 + d * 16:g * 32 + d * 16 + 16],
                                    w2t[:, d * 512 + g * 128:d * 512 + (g + 1) * 128],
                                    U[:, (s - 1) * 32 + d * 16:(s - 1) * 32 + d * 16 + 16],
                                    start=False, stop=(d == 1 and g == 3),
                                    skip_group_check=True)
                    # spread next group's input matmuls into chain slack
                    busy_proj = bool(proj_ready.get(s))
                    ndrain = 0 if busy_proj else (3 if pos < 9 else 2)
                    for _ in range(min(ndrain, len(pending))):
                        inst = pending.pop(0)()
                        if anchor is not None and inst is not None:
                            from concourse.tile import add_dep_helper
                            add_dep_helper(inst.ins, anchor.ins, sync=False,
                                           reason="spread gx fills")

                    act_next = ap_pool.tile([128, 160], f32, tag="act", name="act")
                    # all four gates in ONE tanh (i,f,o preacts prescaled 0.5)
                    nc.scalar.activation(act_cur[:, 0:128], slot[:, 0:128], TANH)
                    tmp = smp.tile([128, 64], f32, tag="tmp", name="tmp")
                    # tmp = ([ti|tf]+1)*[g|c~] = [2*i*g | 4*f*c]
                    nc.vector.scalar_tensor_tensor(
                        tmp[:, :], act_cur[:, 0:64], 1.0, act_cur[:, 96:160],
                        mybir.AluOpType.add, mybir.AluOpType.mult)
                    # c~' = 2c' = 0.5*(4fc) + 2ig
                    nc.vector.scalar_tensor_tensor(
                        act_next[:, 128:160], tmp[:, 32:64], 0.5, tmp[:, 0:32],
                        mybir.AluOpType.mult, mybir.AluOpType.add)
                    tch = smp.tile([128, 32], f32, tag="tch", name="tch")
                    nc.scalar.activation(tch[:, :], act_next[:, 128:160], TANH,
                                         scale=0.5)
                    # u' = (to+1)*tanh(c)  (bf16, feeds next matmuls)
                    nc.vector.scalar_tensor_tensor(
                        U[:, s * 32:(s + 1) * 32], act_cur[:, 64:96], 1.0,
                        tch[:, :],
                        mybir.AluOpType.add, mybir.AluOpType.mult)
                    act_cur = act_next

                    for (d, c) in proj_ready.get(s, ()):
                        emit_proj(d, c)
                    if l == 1:
                        for c in dma_ready.get(s, ()):
                            nc.sync.dma_start(
                                out=dr_out[:, c * CH * BC:(c + 1) * CH * BC],
                                in_=H1[:, c * CH * BC:(c + 1) * CH * BC])
                for (d, c) in post_proj:
                    emit_proj(d, c)
                if l == 1:
                    for c in post_dma:
                        nc.sync.dma_start(
                            out=dr_out[:, c * CH * BC:(c + 1) * CH * BC],
                            in_=H1[:, c * CH * BC:(c + 1) * CH * BC])

    _split_multi_waits(nc, mybir)
    return nc


def _get_nc(seq=SEQ):
    if seq not in _cache:
        _cache[seq] = _build(seq)
    return _cache[seq]


# ---------------------------------------------------------------------------
# Public entry point
# ---------------------------------------------------------------------------
def kernel(x, embedding, params):
    from concourse.bass_utils import run_bass_kernel_spmd

    x = np.asarray(x)
    emb = np.asarray(embedding, np.float32)
    wpack = _host_pack(emb, params)

    emb_x = emb[x]                     # (512, 128, 64)
    in_maps = []
    for c in range(NCORES):
        xc = emb_x[:, c * BC:(c + 1) * BC, :]          # (512, 16, 64)
        x_fm = _bf(xc.transpose(2, 0, 1).reshape(EMBED, SEQ * BC))
        m = {"x0": x_fm}
        m.update(wpack)
        in_maps.append(m)

    nc = _get_nc(SEQ)
    res = run_bass_kernel_spmd(nc, in_maps, core_ids=list(range(NCORES)))

    outs = []
    for c in range(NCORES):
        H1 = res.results[c]["out"]                     # (128, 8192) f32
        outs.append(H1.reshape(HID, SEQ, BC).transpose(1, 2, 0))
    return np.concatenate(outs, axis=1).astype(np.float32)  # (512, 128, 128)


# revision 13
# speedup vs baseline: 1.0492x; 1.0000x over previous
"""Trainium2 Bass kernel for a 2-layer bidirectional projected LSTM encoder.

Problem: x (512, 128) int32 tokens -> embedding (30, 64) -> 2 layers of
bidirectional LSTM (hidden 128, proj 64) -> output (512, 128, 128) f32.

Strategy (per spec sharding hint): data-parallel over batch, 16 batch
elements per NeuronCore, weights replicated. Per core, the two directions
of a layer are fused into shared instructions (feature-major layout, 128
partitions = hidden unit, free dim = [gate|dir|batch]).

Key device-side structure per layer:
  - "gx" input contributions Wih@x + bias are computed by batched matmuls
    directly into per-timestep PSUM slots (4 slots/bank, 3-bank rotating
    groups), bias added via a rank-8 matmul (lhsT=(8,128) bias table,
    rhs=(8,512) one-hot pattern).
  - recurrent contribution uses the merged weight W2 = Whh @ Whr so the
    recurrence runs on u = o*tanh(c) (128-dim) and the output projection
    h = Whr@u moves OFF the critical path (batched every 32 steps).
  - per step: 8 accumulate matmuls (4 gates x 2 dirs) -> sigmoid/tanh on
    ACT straight out of PSUM -> c update on DVE (packed [i|f]*[g|c]) ->
    tanh(c) -> u, which feeds the next step's matmuls.
  - backward direction shares every instruction with forward; its time
    reversal is handled with negative-stride access patterns on the gx
    matmul rhs and projection rhs.

All matmul operands are bf16 (weights preprocessed on host, fp32 PSUM
accumulation, fp32 elementwise), which measured ~3.5e-3 scale-relative
absmax against the fp32 reference.

Measured on 8 axon-tunneled TRN2 cores: HW exec ~2.11 ms, steady-state
~2.02 us per fused (fwd+bwd) timestep; the chain per step is
8 accumulate-matmuls (~450 ns incl. sem) -> tanh ACT (~370) -> 2 fused
scalar_tensor_tensor DVE ops (~450) -> tanh(c) ACT (~355) -> u' STT
(~245), all latency-bound (engines ~30-50% occupied).
"""

import numpy as np
import ml_dtypes

BF = ml_dtypes.bfloat16
SEQ = 512          # sequence length
BC = 16            # batch per core
NCORES = 8
EMBED = 64
HID = 128
PROJ = 64
GS = 12            # steps per PSUM group (3 banks x 4 slots)
CH = 32            # proj chunk: steps per output-projection matmul
PERM = [0, 1, 3, 2]  # reference gate order i,f,g,o -> slot order i,f,o,g

_cache = {}


# ---------------------------------------------------------------------------
# BIR post-fix: this container's walrus encodes at most one semaphore wait
# per TPB_CTRL (Drain/EventSemaphore) instruction; Tile's kernel-tail drain
# aggregates several. Split the extra waits onto fresh single-wait Drains.
# ---------------------------------------------------------------------------
def _split_multi_waits(nc, mybir, limit=1):
    n = [0]

    def fresh():
        n[0] += 1
        return f"I-waitsplit-{n[0]}"

    for fn in nc.m.functions:
        for blk in fn.blocks:
            out = []
            for ins in blk.instructions:
                si = getattr(ins, "sync_info", None)
                if si is not None and si.on_wait and len(si.on_wait) > limit:
                    waits = list(si.on_wait)
                    for w in waits[limit:]:
                        out.append(mybir.InstDrain(
                            name=fresh(), engine=ins.engine, debug=ins.debug,
                            ins=[], outs=[],
                            sync_info=mybir.SyncInfo(on_wait=[w], on_update=[]),
                        ))
                    si.on_wait = waits[:limit]
                out.append(ins)
            blk.instructions = out


# ---------------------------------------------------------------------------
# Host-side weight preprocessing
# ---------------------------------------------------------------------------
def _bf(x):
    return np.ascontiguousarray(np.asarray(x, dtype=np.float32).astype(BF))


def _perm_rows(m):
    return np.concatenate([m[g * HID:(g + 1) * HID] for g in PERM], axis=0)


def _pack_dir(p):
    Wih = np.asarray(p["Wih"], np.float32)
    Whh = np.asarray(p["Whh"], np.float32)
    bias = np.asarray(p["bih"], np.float32) + np.asarray(p["bhh"], np.float32)
    Whr = np.asarray(p["Whr"], np.float32)
    W2 = Whh @ Whr
    # sigmoid(x) = (tanh(x/2)+1)/2: prescale the i,f,o gate rows (slot
    # gates 0..2) by 0.5 so ONE tanh ACT op covers all four gates; the
    # (t+1)/2 affine is fused into the DVE multiplies downstream.
    scl = np.ones((512, 1), np.float32)
    scl[:384] = 0.5
    # state conventions: u is stored as u' = 2u = (tanh(o/2)+1)*tanh(c),
    # so W2 and Whr absorb an extra 0.5 on their u-columns.
    Wp, W2p = _perm_rows(Wih) * scl, _perm_rows(W2) * scl * 0.5
    bp = _perm_rows(bias[:, None])[:, 0] * scl[:, 0]
    return {
        "WihT": Wp.T,                         # (in, 512)
        "bias": bp,                           # (512,)
        "W2T": W2p.T,                         # (128, 512)
        "WhrT": Whr.T * 0.5,                  # (128, 64)
    }


def _host_pack(embedding, params):
    """Build all replicated weight arrays (bf16) once."""
    out = {}
    for l in range(2):
        pk = {d: _pack_dir(params[l][d]) for d in ("fwd", "bwd")}
        out[f"wih{l}"] = _bf(np.concatenate(
            [pk["fwd"]["WihT"], pk["bwd"]["WihT"]], axis=1))      # (in, 1024)
        out[f"w2{l}"] = _bf(np.concatenate(
            [pk["fwd"]["W2T"], pk["bwd"]["W2T"]], axis=1))        # (128, 1024)
        out[f"whr{l}"] = _bf(np.concatenate(
            [pk["fwd"]["WhrT"], pk["bwd"]["WhrT"]], axis=1))      # (128, 128)
        # rank-8 bias table: row j=(g*2+d) -> bias_d[g*128 + p]
        b8 = np.zeros((8, HID), np.float32)
        for g in range(4):
            for d, dn in enumerate(("fwd", "bwd")):
                b8[g * 2 + d] = pk[dn]["bias"][g * HID:(g + 1) * HID]
        out[f"b8{l}"] = _bf(b8)
    # one-hot pattern (8, 512): col n = slot(4)*128 + g*32 + d*16 + b
    e8 = np.zeros((8, 512), np.float32)
    for sl in range(4):
        for g in range(4):
            for d in range(2):
                e8[g * 2 + d, sl * 128 + g * 32 + d * 16:
                   sl * 128 + g * 32 + (d + 1) * 16] = 1.0
    out["e8"] = _bf(e8)
    return out


# ---------------------------------------------------------------------------
# Device program
# ---------------------------------------------------------------------------
def _build(seq=SEQ):
    import concourse.bass as bass
    import concourse.mybir as mybir
    from concourse.tile import TileContext

    f32, bf16 = mybir.dt.float32, mybir.dt.bfloat16
    SIG = mybir.ActivationFunctionType.Sigmoid
    TANH = mybir.ActivationFunctionType.Tanh
    TOK = seq * BC

    nc = bass.Bass()
    dr_x0 = nc.dram_tensor("x0", [EMBED, TOK], bf16, kind="ExternalInput")
    dr_w = {}
    for l, kin in ((0, EMBED), (1, HID)):
        dr_w[f"wih{l}"] = nc.dram_tensor(f"wih{l}", [kin, 1024], bf16, kind="ExternalInput")
        dr_w[f"w2{l}"] = nc.dram_tensor(f"w2{l}", [128, 1024], bf16, kind="ExternalInput")
        dr_w[f"whr{l}"] = nc.dram_tensor(f"whr{l}", [128, 128], bf16, kind="ExternalInput")
        dr_w[f"b8{l}"] = nc.dram_tensor(f"b8{l}", [8, 128], bf16, kind="ExternalInput")
    dr_w["e8"] = nc.dram_tensor("e8", [8, 512], bf16, kind="ExternalInput")
    dr_out = nc.dram_tensor("out", [HID, TOK], f32, kind="ExternalOutput")

    n_groups = (seq + GS - 1) // GS
    n_chunks = seq // CH

    with TileContext(nc) as tc:
        with tc.tile_pool(name="const", bufs=1) as cp, \
             tc.tile_pool(name="state", bufs=1) as sp, \
             tc.tile_pool(name="act", bufs=4) as ap_pool, \
             tc.tile_pool(name="small", bufs=3) as smp, \
             tc.tile_pool(name="gx", bufs=2, space="PSUM") as gxp, \
             tc.tile_pool(name="proj", bufs=2, space="PSUM") as prp:

            # ---- load constants -------------------------------------------
            x0 = cp.tile([EMBED, TOK], bf16, tag="x0", name="x0t")
            nc.sync.dma_start(out=x0[:, :], in_=dr_x0[:, :])
            w = {}
            for l, kin in ((0, EMBED), (1, HID)):
                w[f"wih{l}"] = cp.tile([kin, 1024], bf16, tag=f"wih{l}", name=f"wih{l}")
                w[f"w2{l}"] = cp.tile([128, 1024], bf16, tag=f"w2{l}", name=f"w2{l}")
                w[f"whr{l}"] = cp.tile([128, 128], bf16, tag=f"whr{l}", name=f"whr{l}")
                w[f"b8{l}"] = cp.tile([8, 128], bf16, tag=f"b8{l}", name=f"b8{l}")
            w["e8"] = cp.tile([8, 512], bf16, tag="e8", name="e8")
            for k, t in w.items():
                nc.sync.dma_start(out=t[:, :], in_=dr_w[k][:, :])

            # warm the sigmoid/tanh activation table set early
            warm = cp.tile([128, 8], f32, tag="warm", name="warm")
            nc.gpsimd.memset(warm[:, :], 0.0)
            nc.scalar.activation(warm[:, 0:8], warm[:, 0:8], TANH)


            H0 = sp.tile([HID, TOK], bf16, tag="H0", name="H0")
            H1 = sp.tile([HID, TOK], f32, tag="H1", name="H1")

            for l in range(2):
                X = x0 if l == 0 else H0
                H = H0 if l == 0 else H1
                wih, w2t = w[f"wih{l}"], w[f"w2{l}"]
                whr, b8 = w[f"whr{l}"], w[f"b8{l}"]
                e8 = w["e8"]
                X3 = X.rearrange("p (t b) -> p t b", b=BC)

                U = sp.tile([HID, seq * 2 * BC], bf16, tag="U", name="U")
                U3 = U.rearrange("p (s u) -> p s u", u=2 * BC)

                # gx matmul emitters: group -> list of thunks (banked)
                def gx_mms(k, gt):
                    s_base = k * GS
                    gsteps = min(GS, seq - s_base)
                    nbank = gsteps // 4
                    thunks = []
                    for bank in range(nbank):
                        s0 = s_base + bank * 4
                        gtb = gt[:, bank * 512:(bank + 1) * 512].rearrange(
                            "p (sl c) -> p sl c", c=128)

                        def mk(d, g, s0=s0, gtb=gtb, bank=bank, first=False):
                            def run():
                                o = gtb[:, :, g * 32 + d * 16:g * 32 + d * 16 + 16]
                                if d == 0:
                                    rhs = X3[:, s0:s0 + 4, :]
                                else:
                                    hi = seq - 1 - s0
                                    lo = hi - 4
                                    rhs = (X3[:, hi::-1, :] if lo < 0
                                           else X3[:, hi:lo:-1, :])
                                return nc.tensor.matmul(
                                    o, wih[:, d * 512 + g * 128:d * 512 + (g + 1) * 128],
                                    rhs, start=first, stop=False,
                                    skip_group_check=True)
                            return run

                        for idx, (d, g) in enumerate(
                                [(d, g) for d in (0, 1) for g in range(4)]):
                            thunks.append(mk(d, g, first=(idx == 0)))

                        def bias_mm(gt=gt, bank=bank):
                            return nc.tensor.matmul(
                                gt[:, bank * 512:(bank + 1) * 512],
                                b8[:, :], e8[:, :],
                                start=False, stop=False, skip_group_check=True)
                        thunks.append(bias_mm)
                    return thunks

                # output projection chunks: (dir, chunk) ready at step
                proj_ready = {}
                dma_ready = {}
                post_proj = []
                post_dma = []
                for c in range(n_chunks):
                    s_f = c * CH + CH - 1 + 2   # +2: keep proj MMs out of the
                    s_b = seq - 1 - c * CH + 2  # recurrence-critical PE window
                    if s_f < seq:
                        proj_ready.setdefault(s_f, []).append((0, c))
                    else:
                        post_proj.append((0, c))
                    if s_b < seq:
                        proj_ready.setdefault(s_b, []).append((1, c))
                    else:
                        post_proj.append((1, c))
                    if l == 1:
                        s_d = max(s_f, s_b) + 1
                        if s_d < seq:
                            dma_ready.setdefault(s_d, []).append(c)
                        else:
                            post_dma.append(c)

                def emit_proj(d, c):
                    pt = prp.tile([PROJ, CH * BC], f32, tag="proj", name="proj")
                    if d == 0:
                        rhs = U3[:, c * CH:(c + 1) * CH, 0:BC]
                    else:
                        hi = seq - 1 - c * CH
                        lo = hi - CH
                        rhs = (U3[:, hi::-1, BC:2 * BC] if lo < 0
                               else U3[:, hi:lo:-1, BC:2 * BC])
                    nc.tensor.matmul(pt[:, :], whr[:, d * PROJ:(d + 1) * PROJ],
                                     rhs, start=True, stop=True,
                                     skip_group_check=True)
                    nc.scalar.copy(
                        H[d * PROJ:(d + 1) * PROJ, c * CH * BC:(c + 1) * CH * BC],
                        pt[:, :])

                # ---- the time loop ------------------------------------
                act_cur = ap_pool.tile([128, 160], f32, tag="act", name="act")
                nc.gpsimd.memset(act_cur[:, 128:160], 0.0)

                gt_tiles = {}
                gt_tiles[0] = gxp.tile([128, 512 * ((min(GS, seq) // 4))], f32, tag="gx", name="gx")
                for th in gx_mms(0, gt_tiles[0]):
                    th()
                pending = []   # thunks for next group, drained a few per step

                for s in range(seq):
                    k, pos = divmod(s, GS)
                    if pos == 0:
                        for th in pending:  # flush any leftover gx fills
                            th()
                        pending = []
                        # queue next group's gx fill, spread over early steps
                        if k + 1 < n_groups:
                            nb = (min(GS, seq - (k + 1) * GS)) // 4
                            gt_tiles[k + 1] = gxp.tile([128, 512 * nb], f32, tag="gx", name="gx")
                            pending = gx_mms(k + 1, gt_tiles[k + 1])
                        if k - 1 in gt_tiles:
                            del gt_tiles[k - 1]
                    gt = gt_tiles[k]
                    slot = gt[:, pos * 128:(pos + 1) * 128]

                    # recurrent matmuls accumulate onto gx+bias in PSUM
                    anchor = None
                    if s > 0:
                        for d in (0, 1):
                            for g in range(4):
                                anchor = nc.tensor.matmul(
                                    slot[:, g * 32 + d * 16:g * 32 + d * 16 + 16],
                                    w2t[:, d * 512 + g * 128:d * 512 + (g + 1) * 128],
                                    U[:, (s - 1) * 32 + d * 16:(s - 1) * 32 + d * 16 + 16],
                                    start=False, stop=(d == 1 and g == 3),
                                    skip_group_check=True)
                    # spread next group's input matmuls into chain slack
                    busy_proj = bool(proj_ready.get(s))
                    ndrain = 0 if busy_proj else (3 if pos < 9 else 2)
                    for _ in range(min(ndrain, len(pending))):
                        inst = pending.pop(0)()
                        if anchor is not None and inst is not None:
                            from concourse.tile import add_dep_helper
                            add_dep_helper(inst.ins, anchor.ins, sync=False,
                                           reason="spread gx fills")

                    act_next = ap_pool.tile([128, 160], f32, tag="act", name="act")
                    # all four gates in ONE tanh (i,f,o preacts prescaled 0.5)
                    nc.scalar.activation(act_cur[:, 0:128], slot[:, 0:128], TANH)
                    tmp = smp.tile([128, 64], f32, tag="tmp", name="tmp")
                    # tmp = ([ti|tf]+1)*[g|c~] = [2*i*g | 4*f*c]
                    nc.vector.scalar_tensor_tensor(
                        tmp[:, :], act_cur[:, 0:64], 1.0, act_cur[:, 96:160],
                        mybir.AluOpType.add, mybir.AluOpType.mult)
                    # c~' = 2c' = 0.5*(4fc) + 2ig
                    nc.vector.scalar_tensor_tensor(
                        act_next[:, 128:160], tmp[:, 32:64], 0.5, tmp[:, 0:32],
                        mybir.AluOpType.mult, mybir.AluOpType.add)
                    tch = smp.tile([128, 32], f32, tag="tch", name="tch")
                    nc.scalar.activation(tch[:, :], act_next[:, 128:160], TANH,
                                         scale=0.5)
                    # u' = (to+1)*tanh(c)  (bf16, feeds next matmuls)
                    nc.vector.scalar_tensor_tensor(
                        U[:, s * 32:(s + 1) * 32], act_cur[:, 64:96], 1.0,
                        tch[:, :],
                        mybir.AluOpType.add, mybir.AluOpType.mult)
                    act_cur = act_next

                    for (d, c) in proj_ready.get(s, ()):
                        emit_proj(d, c)
                    if l == 1:
                        for c in dma_ready.get(s, ()):
                            nc.sync.dma_start(
                                out=dr_out[:, c * CH * BC:(c + 1) * CH * BC],
                                in_=H1[:, c * CH * BC:(c + 1) * CH * BC])
                for (d, c) in post_proj:
                    emit_proj(d, c)
                if l == 1:
                    for c in post_dma:
                        nc.sync.dma_start(
                            out=dr_out[:, c * CH * BC:(c + 1) * CH * BC],
                            in_=H1[:, c * CH * BC:(c + 1) * CH * BC])

    _split_multi_waits(nc, mybir)
    return nc


def _get_nc(seq=SEQ):
    if seq not in _cache:
        _cache[seq] = _build(seq)
    return _cache[seq]


# ---------------------------------------------------------------------------
# Public entry point
# ---------------------------------------------------------------------------
def kernel(x, embedding, params):
    from concourse.bass_utils import run_bass_kernel_spmd

    x = np.asarray(x)
    emb = np.asarray(embedding, np.float32)
    wpack = _host_pack(emb, params)

    emb_x = emb[x]                     # (512, 128, 64)
    in_maps = []
    for c in range(NCORES):
        xc = emb_x[:, c * BC:(c + 1) * BC, :]          # (512, 16, 64)
        x_fm = _bf(xc.transpose(2, 0, 1).reshape(EMBED, SEQ * BC))
        m = {"x0": x_fm}
        m.update(wpack)
        in_maps.append(m)

    nc = _get_nc(SEQ)
    res = run_bass_kernel_spmd(nc, in_maps, core_ids=list(range(NCORES)))

    outs = []
    for c in range(NCORES):
        H1 = res.results[c]["out"]                     # (128, 8192) f32
        outs.append(H1.reshape(HID, SEQ, BC).transpose(1, 2, 0))
    return np.concatenate(outs, axis=1).astype(np.float32)  # (512, 128, 128)


# revision 14
# speedup vs baseline: 1.0492x; 1.0000x over previous
"""Trainium2 Bass kernel for a 2-layer bidirectional projected LSTM encoder.

Problem: x (512, 128) int32 tokens -> embedding (30, 64) -> 2 layers of
bidirectional LSTM (hidden 128, proj 64) -> output (512, 128, 128) f32.

Strategy (per spec sharding hint): data-parallel over batch, 16 batch
elements per NeuronCore, weights replicated. Per core, the two directions
of a layer are fused into shared instructions (feature-major layout, 128
partitions = hidden unit, free dim = [gate|dir|batch]).

Key device-side structure per layer:
  - "gx" input contributions Wih@x + bias are computed by batched matmuls
    directly into per-timestep PSUM slots (4 slots/bank, 3-bank rotating
    groups), bias added via a rank-8 matmul (lhsT=(8,128) bias table,
    rhs=(8,512) one-hot pattern).
  - recurrent contribution uses the merged weight W2 = Whh @ Whr so the
    recurrence runs on u = o*tanh(c) (128-dim) and the output projection
    h = Whr@u moves OFF the critical path (batched every 32 steps).
  - per step: 8 accumulate matmuls (4 gates x 2 dirs) -> sigmoid/tanh on
    ACT straight out of PSUM -> c update on DVE (packed [i|f]*[g|c]) ->
    tanh(c) -> u, which feeds the next step's matmuls.
  - backward direction shares every instruction with forward; its time
    reversal is handled with negative-stride access patterns on the gx
    matmul rhs and projection rhs.

All matmul operands are bf16 (weights preprocessed on host, fp32 PSUM
accumulation, fp32 elementwise), which measured ~3.5e-3 scale-relative
absmax against the fp32 reference.

Measured on 8 axon-tunneled TRN2 cores: HW exec ~2.11 ms, steady-state
~2.02 us per fused (fwd+bwd) timestep; the chain per step is
8 accumulate-matmuls (~450 ns incl. sem) -> tanh ACT (~370) -> 2 fused
scalar_tensor_tensor DVE ops (~450) -> tanh(c) ACT (~355) -> u' STT
(~245), all latency-bound (engines ~30-50% occupied).
"""

import numpy as np
import ml_dtypes

BF = ml_dtypes.bfloat16
SEQ = 512          # sequence length
BC = 16            # batch per core
NCORES = 8
EMBED = 64
HID = 128
PROJ = 64
GS = 12            # steps per PSUM group (3 banks x 4 slots)
CH = 32            # proj chunk: steps per output-projection matmul
PERM = [0, 1, 3, 2]  # reference gate order i,f,g,o -> slot order i,f,o,g

_cache = {}


# ---------------------------------------------------------------------------
# BIR post-fix: this container's walrus encodes at most one semaphore wait
# per TPB_CTRL (Drain/EventSemaphore) instruction; Tile's kernel-tail drain
# aggregates several. Split the extra waits onto fresh single-wait Drains.
# ---------------------------------------------------------------------------
def _split_multi_waits(nc, mybir, limit=1):
    n = [0]

    def fresh():
        n[0] += 1
        return f"I-waitsplit-{n[0]}"

    for fn in nc.m.functions:
        for blk in fn.blocks:
            out = []
            for ins in blk.instructions:
                si = getattr(ins, "sync_info", None)
                if si is not None and si.on_wait and len(si.on_wait) > limit:
                    waits = list(si.on_wait)
                    for w in waits[limit:]:
                        out.append(mybir.InstDrain(
                            name=fresh(), engine=ins.engine, debug=ins.debug,
                            ins=[], outs=[],
                            sync_info=mybir.SyncInfo(on_wait=[w], on_update=[]),
                        ))
                    si.on_wait = waits[:limit]
                out.append(ins)
            blk.instructions = out


# ---------------------------------------------------------------------------
# Host-side weight preprocessing
# ---------------------------------------------------------------------------
def _bf(x):
    return np.ascontiguousarray(np.asarray(x, dtype=np.float32).astype(BF))


def _perm_rows(m):
    return np.concatenate([m[g * HID:(g + 1) * HID] for g in PERM], axis=0)


def _pack_dir(p):
    Wih = np.asarray(p["Wih"], np.float32)
    Whh = np.asarray(p["Whh"], np.float32)
    bias = np.asarray(p["bih"], np.float32) + np.asarray(p["bhh"], np.float32)
    Whr = np.asarray(p["Whr"], np.float32)
    W2 = Whh @ Whr
    # sigmoid(x) = (tanh(x/2)+1)/2: prescale the i,f,o gate rows (slot
    # gates 0..2) by 0.5 so ONE tanh ACT op covers all four gates; the
    # (t+1)/2 affine is fused into the DVE multiplies downstream.
    scl = np.ones((512, 1), np.float32)
    scl[:384] = 0.5
    # state conventions: u is stored as u' = 2u = (tanh(o/2)+1)*tanh(c),
    # so W2 and Whr absorb an extra 0.5 on their u-columns.
    Wp, W2p = _perm_rows(Wih) * scl, _perm_rows(W2) * scl * 0.5
    bp = _perm_rows(bias[:, None])[:, 0] * scl[:, 0]
    return {
        "WihT": Wp.T,                         # (in, 512)
        "bias": bp,                           # (512,)
        "W2T": W2p.T,                         # (128, 512)
        "WhrT": Whr.T * 0.5,                  # (128, 64)
    }


def _host_pack(embedding, params):
    """Build all replicated weight arrays (bf16) once."""
    out = {}
    for l in range(2):
        pk = {d: _pack_dir(params[l][d]) for d in ("fwd", "bwd")}
        out[f"wih{l}"] = _bf(np.concatenate(
            [pk["fwd"]["WihT"], pk["bwd"]["WihT"]], axis=1))      # (in, 1024)
        out[f"w2{l}"] = _bf(np.concatenate(
            [pk["fwd"]["W2T"], pk["bwd"]["W2T"]], axis=1))        # (128, 1024)
        out[f"whr{l}"] = _bf(np.concatenate(
            [pk["fwd"]["WhrT"], pk["bwd"]["WhrT"]], axis=1))      # (128, 128)
        # rank-8 bias table: row j=(g*2+d) -> bias_d[g*128 + p]
        b8 = np.zeros((8, HID), np.float32)
        for g in range(4):
            for d, dn in enumerate(("fwd", "bwd")):
                b8[g * 2 + d] = pk[dn]["bias"][g * HID:(g + 1) * HID]
        out[f"b8{l}"] = _bf(b8)
    # one-hot pattern (8, 512): col n = slot(4)*128 + g*32 + d*16 + b
    e8 = np.zeros((8, 512), np.float32)
    for sl in range(4):
        for g in range(4):
            for d in range(2):
                e8[g * 2 + d, sl * 128 + g * 32 + d * 16:
                   sl * 128 + g * 32 + (d + 1) * 16] = 1.0
    out["e8"] = _bf(e8)
    return out


# ---------------------------------------------------------------------------
# Device program
# ---------------------------------------------------------------------------
def _build(seq=SEQ):
    import concourse.bass as bass
    import concourse.mybir as mybir
    from concourse.tile import TileContext

    f32, bf16 = mybir.dt.float32, mybir.dt.bfloat16
    SIG = mybir.ActivationFunctionType.Sigmoid
    TANH = mybir.ActivationFunctionType.Tanh
    TOK = seq * BC

    nc = bass.Bass()
    dr_x0 = nc.dram_tensor("x0", [EMBED, TOK], bf16, kind="ExternalInput")
    dr_w = {}
    for l, kin in ((0, EMBED), (1, HID)):
        dr_w[f"wih{l}"] = nc.dram_tensor(f"wih{l}", [kin, 1024], bf16, kind="ExternalInput")
        dr_w[f"w2{l}"] = nc.dram_tensor(f"w2{l}", [128, 1024], bf16, kind="ExternalInput")
        dr_w[f"whr{l}"] = nc.dram_tensor(f"whr{l}", [128, 128], bf16, kind="ExternalInput")
        dr_w[f"b8{l}"] = nc.dram_tensor(f"b8{l}", [8, 128], bf16, kind="ExternalInput")
    dr_w["e8"] = nc.dram_tensor("e8", [8, 512], bf16, kind="ExternalInput")
    dr_out = nc.dram_tensor("out", [HID, TOK], f32, kind="ExternalOutput")

    n_groups = (seq + GS - 1) // GS
    n_chunks = seq // CH

    with TileContext(nc) as tc:
        with tc.tile_pool(name="const", bufs=1) as cp, \
             tc.tile_pool(name="state", bufs=1) as sp, \
             tc.tile_pool(name="act", bufs=8) as ap_pool, \
             tc.tile_pool(name="small", bufs=8) as smp, \
             tc.tile_pool(name="gx", bufs=2, space="PSUM") as gxp, \
             tc.tile_pool(name="proj", bufs=2, space="PSUM") as prp:

            # ---- load constants -------------------------------------------
            x0 = cp.tile([EMBED, TOK], bf16, tag="x0", name="x0t")
            nc.sync.dma_start(out=x0[:, :], in_=dr_x0[:, :])
            w = {}
            for l, kin in ((0, EMBED), (1, HID)):
                w[f"wih{l}"] = cp.tile([kin, 1024], bf16, tag=f"wih{l}", name=f"wih{l}")
                w[f"w2{l}"] = cp.tile([128, 1024], bf16, tag=f"w2{l}", name=f"w2{l}")
                w[f"whr{l}"] = cp.tile([128, 128], bf16, tag=f"whr{l}", name=f"whr{l}")
                w[f"b8{l}"] = cp.tile([8, 128], bf16, tag=f"b8{l}", name=f"b8{l}")
            w["e8"] = cp.tile([8, 512], bf16, tag="e8", name="e8")
            for k, t in w.items():
                nc.sync.dma_start(out=t[:, :], in_=dr_w[k][:, :])

            # warm the sigmoid/tanh activation table set early
            warm = cp.tile([128, 8], f32, tag="warm", name="warm")
            nc.gpsimd.memset(warm[:, :], 0.0)
            nc.scalar.activation(warm[:, 0:8], warm[:, 0:8], TANH)


            H0 = sp.tile([HID, TOK], bf16, tag="H0", name="H0")
            H1 = sp.tile([HID, TOK], f32, tag="H1", name="H1")

            for l in range(2):
                X = x0 if l == 0 else H0
                H = H0 if l == 0 else H1
                wih, w2t = w[f"wih{l}"], w[f"w2{l}"]
                whr, b8 = w[f"whr{l}"], w[f"b8{l}"]
                e8 = w["e8"]
                X3 = X.rearrange("p (t b) -> p t b", b=BC)

                U = sp.tile([HID, seq * 2 * BC], bf16, tag="U", name="U")
                U3 = U.rearrange("p (s u) -> p s u", u=2 * BC)

                # gx matmul emitters: group -> list of thunks (banked)
                def gx_mms(k, gt):
                    s_base = k * GS
                    gsteps = min(GS, seq - s_base)
                    nbank = gsteps // 4
                    thunks = []
                    for bank in range(nbank):
                        s0 = s_base + bank * 4
                        gtb = gt[:, bank * 512:(bank + 1) * 512].rearrange(
                            "p (sl c) -> p sl c", c=128)

                        def mk(d, g, s0=s0, gtb=gtb, bank=bank, first=False):
                            def run():
                                o = gtb[:, :, g * 32 + d * 16:g * 32 + d * 16 + 16]
                                if d == 0:
                                    rhs = X3[:, s0:s0 + 4, :]
                                else:
                                    hi = seq - 1 - s0
                                    lo = hi - 4
                                    rhs = (X3[:, hi::-1, :] if lo < 0
                                           else X3[:, hi:lo:-1, :])
                                return nc.tensor.matmul(
                                    o, wih[:, d * 512 + g * 128:d * 512 + (g + 1) * 128],
                                    rhs, start=first, stop=False,
                                    skip_group_check=True)
                            return run

                        for idx, (d, g) in enumerate(
                                [(d, g) for d in (0, 1) for g in range(4)]):
                            thunks.append(mk(d, g, first=(idx == 0)))

                        def bias_mm(gt=gt, bank=bank):
                            return nc.tensor.matmul(
                                gt[:, bank * 512:(bank + 1) * 512],
                                b8[:, :], e8[:, :],
                                start=False, stop=False, skip_group_check=True)
                        thunks.append(bias_mm)
                    return thunks

                # output projection chunks: (dir, chunk) ready at step
                proj_ready = {}
                dma_ready = {}
                post_proj = []
                post_dma = []
                for c in range(n_chunks):
                    s_f = c * CH + CH - 1 + 2   # +2: keep proj MMs out of the
                    s_b = seq - 1 - c * CH + 2  # recurrence-critical PE window
                    if s_f < seq:
                        proj_ready.setdefault(s_f, []).append((0, c))
                    else:
                        post_proj.append((0, c))
                    if s_b < seq:
                        proj_ready.setdefault(s_b, []).append((1, c))
                    else:
                        post_proj.append((1, c))
                    if l == 1:
                        s_d = max(s_f, s_b) + 1
                        if s_d < seq:
                            dma_ready.setdefault(s_d, []).append(c)
                        else:
                            post_dma.append(c)

                def emit_proj(d, c):
                    pt = prp.tile([PROJ, CH * BC], f32, tag="proj", name="proj")
                    if d == 0:
                        rhs = U3[:, c * CH:(c + 1) * CH, 0:BC]
                    else:
                        hi = seq - 1 - c * CH
                        lo = hi - CH
                        rhs = (U3[:, hi::-1, BC:2 * BC] if lo < 0
                               else U3[:, hi:lo:-1, BC:2 * BC])
                    nc.tensor.matmul(pt[:, :], whr[:, d * PROJ:(d + 1) * PROJ],
                                     rhs, start=True, stop=True,
                                     skip_group_check=True)
                    nc.scalar.copy(
                        H[d * PROJ:(d + 1) * PROJ, c * CH * BC:(c + 1) * CH * BC],
                        pt[:, :])

                # ---- the time loop ------------------------------------
                act_cur = ap_pool.tile([128, 160], f32, tag="act", name="act")
                nc.gpsimd.memset(act_cur[:, 128:160], 0.0)

                gt_tiles = {}
                gt_tiles[0] = gxp.tile([128, 512 * ((min(GS, seq) // 4))], f32, tag="gx", name="gx")
                for th in gx_mms(0, gt_tiles[0]):
                    th()
                pending = []   # thunks for next group, drained a few per step

                for s in range(seq):
                    k, pos = divmod(s, GS)
                    if pos == 0:
                        for th in pending:  # flush any leftover gx fills
                            th()
                        pending = []
                        # queue next group's gx fill, spread over early steps
                        if k + 1 < n_groups:
                            nb = (min(GS, seq - (k + 1) * GS)) // 4
                            gt_tiles[k + 1] = gxp.tile([128, 512 * nb], f32, tag="gx", name="gx")
                            pending = gx_mms(k + 1, gt_tiles[k + 1])
                        if k - 1 in gt_tiles:
                            del gt_tiles[k - 1]
                    gt = gt_tiles[k]
                    slot = gt[:, pos * 128:(pos + 1) * 128]

                    # recurrent matmuls accumulate onto gx+bias in PSUM
                    anchor = None
                    if s > 0:
                        for d in (0, 1):
                            for g in range(4):
                                anchor = nc.tensor.matmul(
                                    slot[:, g * 32 + d * 16:g * 32 + d * 16 + 16],
                                    w2t[:, d * 512 + g * 128:d * 512 + (g + 1) * 128],
                                    U[:, (s - 1) * 32 + d * 16:(s - 1) * 32 + d * 16 + 16],
                                    start=False, stop=(d == 1 and g == 3),
                                    skip_group_check=True)
                    # spread next group's input matmuls into chain slack
                    busy_proj = bool(proj_ready.get(s))
                    ndrain = 0 if busy_proj else (3 if pos < 9 else 2)
                    for _ in range(min(ndrain, len(pending))):
                        inst = pending.pop(0)()
                        if anchor is not None and inst is not None:
                            from concourse.tile import add_dep_helper
                            add_dep_helper(inst.ins, anchor.ins, sync=False,
                                           reason="spread gx fills")

                    act_next = ap_pool.tile([128, 160], f32, tag="act", name="act")
                    # all four gates in ONE tanh (i,f,o preacts prescaled 0.5)
                    nc.scalar.activation(act_cur[:, 0:128], slot[:, 0:128], TANH)
                    tmp = smp.tile([128, 64], f32, tag="tmp", name="tmp")
                    # tmp = ([ti|tf]+1)*[g|c~] = [2*i*g | 4*f*c]
                    nc.vector.scalar_tensor_tensor(
                        tmp[:, :], act_cur[:, 0:64], 1.0, act_cur[:, 96:160],
                        mybir.AluOpType.add, mybir.AluOpType.mult)
                    # c~' = 2c' = 0.5*(4fc) + 2ig
                    nc.vector.scalar_tensor_tensor(
                        act_next[:, 128:160], tmp[:, 32:64], 0.5, tmp[:, 0:32],
                        mybir.AluOpType.mult, mybir.AluOpType.add)
                    tch = smp.tile([128, 32], f32, tag="tch", name="tch")
                    nc.scalar.activation(tch[:, :], act_next[:, 128:160], TANH,
                                         scale=0.5)
                    # u' = (to+1)*tanh(c)  (bf16, feeds next matmuls)
                    nc.vector.scalar_tensor_tensor(
                        U[:, s * 32:(s + 1) * 32], act_cur[:, 64:96], 1.0,
                        tch[:, :],
                        mybir.AluOpType.add, mybir.AluOpType.mult)
                    act_cur = act_next

                    for (d, c) in proj_ready.get(s, ()):
                        emit_proj(d, c)
                    if l == 1:
                        for c in dma_ready.get(s, ()):
                            nc.sync.dma_start(
                                out=dr_out[:, c * CH * BC:(c + 1) * CH * BC],
                                in_=H1[:, c * CH * BC:(c + 1) * CH * BC])
                for (d, c) in post_proj:
                    emit_proj(d, c)
                if l == 1:
                    for c in post_dma:
                        nc.sync.dma_start(
                            out=dr_out[:, c * CH * BC:(c + 1) * CH * BC],
                            in_=H1[:, c * CH * BC:(c + 1) * CH * BC])

    _split_multi_waits(nc, mybir)
    return nc


def _get_nc(seq=SEQ):
    if seq not in _cache:
        _cache[seq] = _build(seq)
    return _cache[seq]


# ---------------------------------------------------------------------------
# Public entry point
# ---------------------------------------------------------------------------
def kernel(x, embedding, params):
    from concourse.bass_utils import run_bass_kernel_spmd

    x = np.asarray(x)
    emb = np.asarray(embedding, np.float32)
    wpack = _host_pack(emb, params)

    emb_x = emb[x]                     # (512, 128, 64)
    in_maps = []
    for c in range(NCORES):
        xc = emb_x[:, c * BC:(c + 1) * BC, :]          # (512, 16, 64)
        x_fm = _bf(xc.transpose(2, 0, 1).reshape(EMBED, SEQ * BC))
        m = {"x0": x_fm}
        m.update(wpack)
        in_maps.append(m)

    nc = _get_nc(SEQ)
    res = run_bass_kernel_spmd(nc, in_maps, core_ids=list(range(NCORES)))

    outs = []
    for c in range(NCORES):
        H1 = res.results[c]["out"]                     # (128, 8192) f32
        outs.append(H1.reshape(HID, SEQ, BC).transpose(1, 2, 0))
    return np.concatenate(outs, axis=1).astype(np.float32)  # (512, 128, 128)
